# revision 37
# baseline (speedup 1.0000x reference)
"""Trainium2 Bass kernel for nn_CF_34016140984494 (dense_cnn).

Data-parallel over batch N=8 across 8 NeuronCores (1 image per core).
Per-core pipeline:
  A: conv1(1x1)+BN1 -> nearest-upsample -> attention fuse          (PE/DVE)
  T: PE-transpose xf to pixel-major, assemble 2x2-patch tensor G   (PE/DMA)
  B: offset conv3x3                                                (PE)
  C: bilinear index + weight pipeline (folded [63,448] layout)     (DVE)
  D: dma_gather 2KB patch rows from G + lerp + DCN matmul          (DMA/DVE/PE)
"""
import numpy as np
from contextlib import ExitStack

N, CIN, MID, OUT, H, W = 8, 512, 256, 256, 56, 56
HY, WY = 28, 28
EPS = 1e-5
K = 9
HP, WP = H + 2, W + 2          # 58x58 zero-padded layout
P = H * W                      # 3136
PP = HP * WP                   # 3364
PPA = PP + 12                  # padded alloc
NT = 7                         # spatial tiles
TS = P // NT                   # 448 pixels per tile
FJ = 7                         # fold factor (row = k*FJ + nt)
FR = K * FJ                    # 63 folded rows
NIDX = 3 * TS                  # per-gather idx count (one ky row group)

_CACHE = {}


# ----------------------------------------------------------------- host math
def _fold_weights(d):
    f = {}
    scale1 = d['bn1_gamma'] / np.sqrt(d['bn1_var'] + EPS)
    bias1 = d['bn1_beta'] - d['bn1_mean'] * scale1
    W1 = (d['conv1_w'] * scale1[:, None]).astype(np.float32)            # [MID, CIN]
    f['w1T'] = np.ascontiguousarray(W1.T)                               # [CIN, MID]
    f['b1'] = bias1.astype(np.float32).reshape(MID, 1)
    Ax = (d['att_w'][:, :MID] @ W1).astype(np.float32)                  # [2, CIN]
    f['axT'] = np.ascontiguousarray(Ax.T)                               # [CIN, 2]
    f['ayT'] = np.ascontiguousarray(d['att_w'][:, MID:].astype(np.float32).T)  # [MID, 2]
    f['ba'] = (d['att_w'][:, :MID] @ bias1 + d['att_b']).astype(np.float32).reshape(2, 1)
    ow = d['off_w'].reshape(2 * K, MID, K)
    perm = np.concatenate([np.arange(0, 18, 2), np.arange(1, 18, 2)])   # dy rows, then dx rows
    f['woffT'] = np.ascontiguousarray(ow[perm].transpose(1, 2, 0)).astype(np.float32)  # [MID, K, 18]
    scale2 = d['bn2_gamma'] / np.sqrt(d['bn2_var'] + EPS)
    W2 = (d['dcn_w'].reshape(OUT, MID * K) * scale2[:, None]).astype(np.float32)
    w2r = W2.reshape(OUT, MID, K).transpose(2, 1, 0).reshape(MID * K, OUT)  # rows k-major c-minor
    f['w2T'] = np.ascontiguousarray(w2r)
    f['b2'] = (d['bn2_beta'] - d['bn2_mean'] * scale2).astype(np.float32).reshape(OUT, 1)
    return f


def _build_consts():
    """Folded base-coordinate tables [63, 448]: row = k*FJ + nt."""
    kv = np.arange(K)
    ky = (kv // 3).astype(np.float32)
    kx = (kv % 3).astype(np.float32)
    p = np.arange(P)
    hh = (p // W).astype(np.float32)
    ww = (p % W).astype(np.float32)
    basey = hh[None, :] - 1.0 + ky[:, None]      # [9, P]
    basex = ww[None, :] - 1.0 + kx[:, None]
    byf = basey.reshape(K, FJ, TS).reshape(FR, TS)
    bxf = basex.reshape(K, FJ, TS).reshape(FR, TS)
    return byf.astype(np.float32), bxf.astype(np.float32)


# ------------------------------------------------------------- device build
def _build_program():
    import concourse.bass as bass
    import concourse.tile as tile
    from concourse import bacc, mybir
    dt = mybir.dt
    AF = mybir.ActivationFunctionType
    OP = mybir.AluOpType

    nc = bacc.Bacc("TRN2", target_bir_lowering=False, debug=False, num_devices=N)

    f32, bf16 = dt.float32, dt.bfloat16
    i16, i32 = dt.int16, dt.int32

    # --- DRAM I/O
    x_im = nc.dram_tensor("x_im", [CIN, P], bf16, kind="ExternalInput").ap()
    y_im = nc.dram_tensor("y_im", [MID, HY * WY], bf16, kind="ExternalInput").ap()
    w1T = nc.dram_tensor("w1T", [CIN, MID], bf16, kind="ExternalInput").ap()
    axT = nc.dram_tensor("axT", [CIN, 2], bf16, kind="ExternalInput").ap()
    ayT = nc.dram_tensor("ayT", [MID, 2], bf16, kind="ExternalInput").ap()
    woffT = nc.dram_tensor("woffT", [MID, K, 18], bf16, kind="ExternalInput").ap()
    w2T = nc.dram_tensor("w2T", [MID * K, OUT], bf16, kind="ExternalInput").ap()
    b1 = nc.dram_tensor("b1", [MID, 1], f32, kind="ExternalInput").ap()
    ba = nc.dram_tensor("ba", [2, 1], f32, kind="ExternalInput").ap()
    b2 = nc.dram_tensor("b2", [OUT, 1], f32, kind="ExternalInput").ap()
    basey = nc.dram_tensor("basey", [FR, TS], f32, kind="ExternalInput").ap()
    basex = nc.dram_tensor("basex", [FR, TS], f32, kind="ExternalInput").ap()
    idm2 = nc.dram_tensor("idm2", [2, 2], f32, kind="ExternalInput").ap()
    out_im = nc.dram_tensor("out_im", [OUT, P], f32, kind="ExternalOutput").ap()
    xf_im = nc.dram_tensor("xf_im", [MID, P], f32, kind="ExternalOutput").ap()
    # internal DRAM scratch
    soff = nc.dram_tensor("soff", [2 * K, P], f32).ap()
    sidx = nc.dram_tensor("sidx", [NT, 3, 16, NIDX // 16], i16).ap()
    # bilinear weights (j-minor), replicated x16 partitions (row = k*FJ + nt)
    w4d16 = nc.dram_tensor("w4d16", [16, FR, TS, 4], bf16).ap()

    def mm(out, lhsT, rhs, start, stop):
        nc.tensor.matmul(out, lhsT, rhs, start=start, stop=stop)

    with tile.TileContext(nc) as tc, ExitStack() as ctx:
        wpool = ctx.enter_context(tc.tile_pool(name="weights", bufs=1))
        tpool = ctx.enter_context(tc.tile_pool(name="tp", bufs=1))
        xfctx = ExitStack()
        xfpool = xfctx.enter_context(tc.tile_pool(name="xfp", bufs=1))
        pipectx = ExitStack()
        pipepool = pipectx.enter_context(tc.tile_pool(name="pipe", bufs=1))

        # ---- persistent weight tiles
        w1S = [wpool.tile([128, MID], bf16, tag=f"w1_{i}", name=f"w1_{i}") for i in range(4)]
        for i in range(4):
            nc.sync.dma_start(w1S[i][:], w1T[i * 128:(i + 1) * 128, :])
        axS = [wpool.tile([128, 2], bf16, tag=f"ax_{i}", name=f"ax_{i}") for i in range(4)]
        for i in range(4):
            nc.sync.dma_start(axS[i][:], axT[i * 128:(i + 1) * 128, :])
        ayS = [wpool.tile([128, 2], bf16, tag=f"ay_{i}", name=f"ay_{i}") for i in range(2)]
        for i in range(2):
            nc.sync.dma_start(ayS[i][:], ayT[i * 128:(i + 1) * 128, :])
        woffS = [wpool.tile([128, K, 18], bf16, tag=f"wo_{i}", name=f"wo_{i}") for i in range(2)]
        for i in range(2):
            nc.sync.dma_start(woffS[i][:], woffT[i * 128:(i + 1) * 128, :, :])
        w2S = [wpool.tile([128, OUT], bf16, tag=f"w2_{i}", name=f"w2_{i}") for i in range(18)]
        for i in range(18):
            nc.sync.dma_start(w2S[i][:], w2T[i * 128:(i + 1) * 128, :])
        b1S = [wpool.tile([128, 1], f32, tag=f"b1_{i}", name=f"b1_{i}") for i in range(2)]
        for i in range(2):
            nc.sync.dma_start(b1S[i][:], b1[i * 128:(i + 1) * 128, :])
        baS = wpool.tile([2, 1], f32, tag="ba")
        nc.sync.dma_start(baS[:], ba[:, :])
        b2S = [wpool.tile([128, 1], f32, tag=f"b2_{i}", name=f"b2_{i}") for i in range(2)]
        for i in range(2):
            nc.sync.dma_start(b2S[i][:], b2[i * 128:(i + 1) * 128, :])
        byS = pipepool.tile([FR, TS], f32, tag="basey")
        nc.sync.dma_start(byS[:], basey[:, :])
        bxS = pipepool.tile([FR, TS], f32, tag="basex")
        nc.sync.dma_start(bxS[:], basex[:, :])
        idm2S = wpool.tile([2, 2], f32, tag="idm2")
        nc.sync.dma_start(idm2S[:], idm2[:, :])
        idxw = wpool.tile([128, NT, 3, NIDX // 16], i16, tag="idxw")

        # ---- padded bf16 xf storage, zeroed borders
        xfb = [xfpool.tile([128, PPA], bf16, tag=f"xfb{cb}", name=f"xfb{cb}")
               for cb in range(2)]
        for cb in range(2):
            nc.vector.memset(xfb[cb][:], 0.0)

        # ================= phase A: conv1 + upsample + attention =========
        with tc.tile_pool(name="phA", bufs=2) as pa, \
             tc.tile_pool(name="phAy", bufs=1) as pay, \
             tc.tile_pool(name="psA", bufs=2, space="PSUM") as psA, \
             tc.tile_pool(name="psZ", bufs=2, space="PSUM") as psZ:
            ySr = [pay.tile([128, HY, WY], bf16, tag=f"y{i}", name=f"y{i}") for i in range(2)]
            for i in range(2):
                nc.sync.dma_start(
                    ySr[i][:], y_im[i * 128:(i + 1) * 128, :].rearrange(
                        "p (a b) -> p a b", a=HY, b=WY))
            xF = [pay.tile([128, P], bf16, tag=f"xF{i}", name=f"xF{i}") for i in range(4)]
            for i in range(4):
                nc.sync.dma_start(xF[i][:], x_im[i * 128:(i + 1) * 128, :])
            for nt in range(NT):
                xs = [xF[i][:, nt * TS:(nt + 1) * TS] for i in range(4)]
                yus = [pa.tile([128, 8, W], bf16, tag=f"yu{i}", name=f"yus{i}_{nt}")
                       for i in range(2)]
                for cb in range(2):
                    ysrc = ySr[cb][:, nt * 4:(nt + 1) * 4, :]
                    for dy in range(2):
                        for dx in range(2):
                            nc.scalar.activation(yus[cb][:, dy::2, dx::2], ysrc,
                                                 AF.Identity)
                # conv1 + BN1 bias
                xms = [pa.tile([128, TS], f32, tag=f"xm{i}", name=f"xms{i}_{nt}")
                       for i in range(2)]
                for mb in range(2):
                    pt = psA.tile([128, TS], f32, tag="c1", name=f"c1_{mb}_{nt}")
                    for kt in range(4):
                        mm(pt[:], w1S[kt][:, mb * 128:(mb + 1) * 128], xs[kt],
                           kt == 0, kt == 3)
                    nc.scalar.activation(xms[mb][:], pt[:], AF.Identity,
                                         bias=b1S[mb][:])
                # attention logits + sigmoid
                pz = psZ.tile([2, TS], f32, tag="zp", name=f"zp_{nt}")
                for kt in range(4):
                    mm(pz[:], axS[kt][:], xs[kt], kt == 0, False)
                for cb in range(2):
                    yuf = yus[cb][:].rearrange("p a b -> p (a b)")
                    mm(pz[:], ayS[cb][:], yuf, False, cb == 1)
                zs = pa.tile([2, TS], f32, tag="zs", name=f"zs_{nt}")
                nc.scalar.activation(zs[:], pz[:], AF.Sigmoid, bias=baS[:])
                # replicate z rows across partitions via one-hot matmul
                zp = [psZ.tile([128, TS], f32, tag=f"zr{i}", name=f"zr{i}_{nt}")
                      for i in range(2)]
                for i in range(2):
                    mm(zp[i][:], idm2S[:, i:i + 1].broadcast_to((2, 128)), zs[:],
                       True, True)
                # xf = xm*z0 + yu*z1
                for cb in range(2):
                    yuf = yus[cb][:].rearrange("p a b -> p (a b)")
                    t0 = pa.tile([128, TS], f32, tag="t0", name=f"t0_{nt}_{cb}")
                    nc.vector.tensor_tensor(t0[:], xms[cb][:], zp[0][:], OP.mult)
                    t1 = pa.tile([128, TS], f32, tag="t1", name=f"t1_{nt}_{cb}")
                    nc.vector.tensor_tensor(t1[:], yuf, zp[1][:], OP.mult)
                    xff = pa.tile([128, TS], f32, tag="xff", name=f"xff_{nt}_{cb}")
                    nc.vector.tensor_tensor(xff[:], t0[:], t1[:], OP.add)
                    dstv = xfb[cb][:, :PP].rearrange("p (a b) -> p a b", a=HP, b=WP)
                    nc.scalar.activation(
                        dstv[:, nt * 8 + 1:nt * 8 + 9, 1:57],
                        xff[:].rearrange("p (a b) -> p a b", a=8, b=W),
                        AF.Identity)
                    nc.sync.dma_start(
                        xf_im[cb * 128:(cb + 1) * 128, nt * TS:(nt + 1) * TS],
                        xff[:])

        # ================= phase T: pack 2x2 patches for ap_gather =======
        # xq[p][pp] = 2x2 patch for channel p (slots 0-3) and p+128 (4-7)
        xq = tpool.tile([128, PPA, 8], bf16, tag="xq")
        for cb in range(2):
            for j, dj in enumerate((0, 1, WP, WP + 1)):
                if j == 0:
                    nc.scalar.activation(xq[:, :PP - dj, cb * 4 + j],
                                         xfb[cb][:, dj:PP], AF.Identity)
                else:
                    nc.vector.tensor_copy(xq[:, :PP - dj, cb * 4 + j],
                                          xfb[cb][:, dj:PP])

        # ================= phase B: offset conv ==========================
        with tc.tile_pool(name="phB", bufs=1) as pb, \
             tc.tile_pool(name="psB", bufs=2, space="PSUM") as psB:
            offS = pb.tile([18, P], f32, tag="off")
            for nt in range(NT):
                po = psB.tile([18, TS], f32, tag="offp", name=f"offp_{nt}")
                first = True
                for kk in range(K):
                    ky, kx = kk // 3, kk % 3
                    for cb in range(2):
                        rhs = xfb[cb][:, :PP].rearrange("p (a b) -> p a b", a=HP, b=WP)[
                            :, nt * 8 + ky:nt * 8 + ky + 8, kx:kx + W]
                        mm(po[:], woffS[cb][:, kk, :], rhs, first,
                           kk == K - 1 and cb == 1)
                        first = False
                nc.vector.tensor_copy(offS[:, nt * TS:(nt + 1) * TS], po[:])
            nc.sync.dma_start(soff[:, :], offS[:])

        # ================= phase C: index/weight pipeline (folded) =======
        sofr = soff.rearrange("r (j s) -> r j s", j=FJ)

        def pp_t(nm):
            return pipepool.tile([FR, TS], f32, tag="pp", name=nm, bufs=12)

        offy = pp_t("offy")
        nc.sync.dma_start(offy[:], sofr[0:K].rearrange("r j s -> (r j) s"))
        offx = pp_t("offx")
        nc.sync.dma_start(offx[:], sofr[K:2 * K].rearrange("r j s -> (r j) s"))

        def floor_pipeline(offT, baseT, tag):
            s = pp_t(f"s_{tag}")
            nc.vector.tensor_tensor(s[:], offT[:], baseT[:], OP.add)
            ri = pipepool.tile([FR, TS], i32, tag="ppi", name=f"ri_{tag}", bufs=2)
            nc.vector.tensor_copy(ri[:], s[:])
            r0 = pp_t(f"r0_{tag}")
            nc.vector.tensor_copy(r0[:], ri[:])
            gt = pp_t(f"gt_{tag}")
            nc.vector.tensor_tensor(gt[:], r0[:], s[:], OP.is_gt)
            fl = pp_t(f"fl_{tag}")
            nc.vector.tensor_tensor(fl[:], r0[:], gt[:], OP.subtract)
            fr = pp_t(f"fr_{tag}")
            nc.vector.tensor_tensor(fr[:], s[:], fl[:], OP.subtract)
            va = pp_t(f"va_{tag}")
            nc.vector.tensor_scalar(va[:], fl[:], 0.0, None, OP.is_ge)
            vb = pp_t(f"vb_{tag}")
            nc.vector.tensor_scalar(vb[:], fl[:], 55.0, None, OP.is_le)
            v0 = pp_t(f"v0_{tag}")
            nc.vector.tensor_tensor(v0[:], va[:], vb[:], OP.mult)
            nc.vector.tensor_scalar(va[:], fl[:], -1.0, None, OP.is_ge)
            nc.vector.tensor_scalar(vb[:], fl[:], 54.0, None, OP.is_le)
            v1 = pp_t(f"v1_{tag}")
            nc.vector.tensor_tensor(v1[:], va[:], vb[:], OP.mult)
            w0 = pp_t(f"w0_{tag}")
            nc.vector.tensor_scalar(w0[:], fr[:], -1.0, 1.0, OP.mult, OP.add)
            nc.vector.tensor_tensor(w0[:], w0[:], v0[:], OP.mult)
            w1 = pp_t(f"w1_{tag}")
            nc.vector.tensor_tensor(w1[:], fr[:], v1[:], OP.mult)
            flc = pp_t(f"flc_{tag}")
            nc.vector.tensor_scalar(flc[:], fl[:], -1.0, 55.0, OP.max, OP.min)
            return flc, w0, w1

        fly, wy0, wy1 = floor_pipeline(offy, byS, "y")
        flx, wx0, wx1 = floor_pipeline(offx, bxS, "x")

        # 4 bilinear weight planes [63, 448, 4] bf16 (j-minor, matches patch)
        w4 = pipepool.tile([FR, TS, 4], bf16, tag="w4")
        nc.vector.tensor_tensor(w4[:, :, 0], wy0[:], wx0[:], OP.mult)
        nc.vector.tensor_tensor(w4[:, :, 1], wy0[:], wx1[:], OP.mult)
        nc.vector.tensor_tensor(w4[:, :, 2], wy1[:], wx0[:], OP.mult)
        nc.vector.tensor_tensor(w4[:, :, 3], wy1[:], wx1[:], OP.mult)
        # scatter to DRAM replicated x16
        for g in range(16):
            nc.sync.dma_start(w4d16[g], w4[:])

        # padded flat index pp0 = (y0c+1)*58 + (x0c+1)
        i0f = pp_t("i0f")
        nc.vector.tensor_scalar(i0f[:], fly[:], float(WP), float(WP + 1),
                                OP.mult, OP.add)
        nc.vector.tensor_tensor(i0f[:], i0f[:], flx[:], OP.add)
        ii = pipepool.tile([FR, TS], i32, tag="ppi", name="ii", bufs=2)
        nc.vector.tensor_copy(ii[:], i0f[:])
        is_ = pipepool.tile([FR, 16, TS // 16], i16, tag="pps", name="is")
        nc.vector.tensor_copy(
            is_[:].rearrange("r q s -> r s q"),
            ii[:].rearrange("r (s q) -> r s q", s=TS // 16, q=16))
        # scatter wrapped idxs to DRAM: per k, rows {k*FJ + nt} -> sidx[nt][ky][kkr]
        for kk in range(K):
            ky, kkr = kk // 3, kk % 3
            nc.sync.dma_start(
                sidx[:, ky, :, kkr * 28:(kkr + 1) * 28],
                is_[kk * FJ:(kk + 1) * FJ, :, :])
        # replicate to 128 partitions (8 groups of 16)
        for g in range(8):
            nc.sync.dma_start(
                idxw[g * 16:(g + 1) * 16],
                sidx.rearrange("nt ky q s -> q nt ky s"))

        # ================= phase D: gather + lerp + DCN matmul ===========
        pipectx.close()  # release pipeline tiles
        xfctx.close()    # release xfb
        with tc.tile_pool(name="gop", bufs=2) as gop, \
             tc.tile_pool(name="w8p", bufs=2) as w8p, \
             tc.tile_pool(name="mp", bufs=1) as mp, \
             tc.tile_pool(name="vp", bufs=2) as vp, \
             tc.tile_pool(name="op", bufs=2) as op_, \
             tc.tile_pool(name="psD", bufs=2, space="PSUM") as psD:
            for nt in range(NT):
                V = vp.tile([128, 2 * K, TS], bf16, tag="V", name=f"V_{nt}")
                for ky in range(3):
                    # weights for the 3 kernel points of this ky, all 128 parts
                    w8 = w8p.tile([128, 3, TS, 4], bf16, tag="w8",
                                  name=f"w8_{nt}_{ky}")
                    w4v = w4d16.rearrange("g (b a) s c -> g b a s c",
                                          b=K, a=NT)[:, 3 * ky:3 * ky + 3, nt]
                    for g in range(8):
                        nc.sync.dma_start(w8[g * 16:(g + 1) * 16], w4v)
                    gt = gop.tile([128, 3, TS, 4], f32, tag="gq",
                                  name=f"gq_{nt}_{ky}")
                    nc.gpsimd.ap_gather(
                        gt[:].rearrange("p a s d -> p (a s) d"),
                        xq[:].bitcast(f32),
                        idxw[:, nt, ky, :], channels=128,
                        num_elems=PPA, d=4, num_idxs=NIDX)
                    gb = gt[:].bitcast(bf16)     # [128, 3, TS, 8]
                    for cb in range(2):
                        m = mp.tile([128, 3, TS, 4], bf16, tag="m",
                                    name=f"m_{nt}_{ky}_{cb}")
                        nc.vector.tensor_tensor(
                            m[:], gb.rearrange("p a s (c j) -> p a s c j", c=2)
                            [:, :, :, cb, :], w8[:], OP.mult)
                        a0 = mp.tile([128, 3, TS, 2], bf16, tag="a0",
                                     name=f"a0_{nt}_{ky}_{cb}")
                        nc.vector.tensor_tensor(a0[:], m[:, :, :, 0::2],
                                                m[:, :, :, 1::2], OP.add)
                        for kkr in range(3):
                            kk = 3 * ky + kkr
                            nc.vector.tensor_tensor(V[:, 2 * kk + cb, :],
                                                    a0[:, kkr, :, 0],
                                                    a0[:, kkr, :, 1], OP.add)
                for mb in range(2):
                    pD = psD.tile([128, TS], f32, tag="dcn", name=f"dcn_{nt}_{mb}")
                    for kt in range(18):
                        nc.tensor.matmul(pD[:], w2S[kt][:, mb * 128:(mb + 1) * 128],
                                         V[:, kt, :], start=(kt == 0), stop=(kt == 17))
                    oS = op_.tile([128, TS], f32, tag="o", name=f"o_{nt}_{mb}")
                    nc.scalar.activation(oS[:], pD[:], AF.Identity, bias=b2S[mb][:])
                    nc.sync.dma_start(out_im[mb * 128:(mb + 1) * 128,
                                             nt * TS:(nt + 1) * TS], oS[:])

    nc.compile()
    return nc


def _in_maps(d):
    import ml_dtypes
    f = _fold_weights(d)
    byf, bxf = _build_consts()
    bf = ml_dtypes.bfloat16
    shared = {
        'w1T': f['w1T'].astype(bf), 'axT': f['axT'].astype(bf),
        'ayT': f['ayT'].astype(bf), 'woffT': f['woffT'].astype(bf),
        'w2T': f['w2T'].astype(bf),
        'b1': f['b1'], 'ba': f['ba'], 'b2': f['b2'],
        'basey': byf, 'basex': bxf,
        'idm2': np.eye(2, 2, dtype=np.float32),
    }
    maps = []
    for n in range(N):
        m = dict(shared)
        m['x_im'] = np.ascontiguousarray(d['x'][n].reshape(CIN, P)).astype(bf)
        m['y_im'] = np.ascontiguousarray(d['y'][n].reshape(MID, HY * WY)).astype(bf)
        maps.append(m)
    return maps


def kernel(**inputs):
    d = {k: np.asarray(v) for k, v in inputs.items()}
    if 'nc' not in _CACHE:
        _CACHE['nc'] = _build_program()
    nc = _CACHE['nc']
    from concourse.bass_utils import run_bass_kernel_spmd
    maps = _in_maps(d)
    res = run_bass_kernel_spmd(nc, maps, list(range(N)))
    _CACHE['last_res'] = res
    outs = np.stack([res.results[i]['out_im'].reshape(OUT, H, W) for i in range(N)])
    xfs = np.stack([res.results[i]['xf_im'].reshape(MID, H, W) for i in range(N)])
    return outs.astype(np.float32), xfs.astype(np.float32)
